# revision 36
# baseline (speedup 1.0000x reference)
"""Trainium2 Bass kernel for nn_DNM_Conv (LayerNorm -> synapse contraction ->
dendritic weighting -> GELU -> residual multiply).

Algebraic reduction of the reference:
    y = LayerNorm(x)                                  (b, n, d)
    t[b,o,d] = sum_n W[o,n] * y[b,n,d] + c[o]
        where W[o,n] = sum_m dw[o,m]*sw[o,m,n],  c[o] = sum_{m,n} dw[o,m]*sb[o,m,n]
    out = x * (gelu_erf(t) + 1)                       (o == n == 196)

Kernel structure (v5):
  * All inputs partition-major; loads ordered tiny-first on the sync queue
    so constants don't queue behind megabyte transfers; stores also on the
    sync HW queue (the gpsimd software queue moves small packets slowly).
  * LN stats via PE: host ships [x^T | (x^2)^T] fp8e4; basis-column matmuls
    reduce over d into [sum x | sum x^2] PSUM rows, two batch-groups of 4.
    Dummy matmuls warm the PE (HAM) during the input DMA.
  * mean/var on [4,196] rows (DVE), both rstd ops back-to-back in one
    sqrt table phase (ACT), rows PE-transposed to per-partition columns.
  * rstd folded into per-batch weights wr = wt * rstd (DVE tensor_scalar);
    the mu term becomes a per-batch gelu bias c - (W @ (rstd*mu)) computed
    by a tiny PE matmul against the transposed z columns.
  * GELU reads both d-chunk PSUM banks of a batch in one instruction.
  * Output: (g+1) in place (DVE 4x) then one tensor_tensor multiply by x
    per (pair, o-chunk); single DRAM write, no residual seed/accumulate.
  * c shipped as a [1, O] row (one DMA descriptor) and PE-transposed.

Distribution: data-parallel over batch, 8 batches per core on 8 cores.
Datapath fp16 (host casts), fp8 stats operands, fp32 PSUM + LN statistics.
"""

import numpy as np

B, N, D, O, M = 64, 196, 768, 196, 2
N_CORES = 8
BPC = B // N_CORES          # batches per core
NPAIR = BPC // 2            # batch pairs
NG = 2                      # stats batch groups
GB = BPC // NG              # batches per group (4)
NA, NB = 128, 68            # n (and o) partition split
DC = 384                    # matmul free-dim chunk (one PSUM bank)
NDCH = D // 128             # d-chunks for the stats matmuls (6)
SQ = 2 * N                  # stats row width: [sum x | sum x^2]
LN_EPS = 1e-5
N_WARM = 18                 # PE warm-up dummy matmuls

_NC_CACHE = {}


def _build_nc(nontrivial_ln):
    import concourse.bacc as bacc
    import concourse.tile as tile
    import concourse.bass as bass
    from concourse.tile import add_dep_helper
    from concourse import mybir
    from contextlib import ExitStack
    import ml_dtypes

    F32 = mybir.dt.float32
    F16 = mybir.dt.float16
    F8 = mybir.dt.float8e4
    AF = mybir.ActivationFunctionType
    OP = mybir.AluOpType

    nc = bacc.Bacc()
    xa_d = nc.declare_dram_parameter("xa", [NA, NPAIR, 2, D], F16, isOutput=False)
    xb_d = nc.declare_dram_parameter("xb", [NB, NPAIR, 2, D], F16, isOutput=False)
    xs_d = nc.declare_dram_parameter("xs", [128, NG, NDCH, GB, SQ], F8,
                                     isOutput=False)
    wt_d = nc.declare_dram_parameter("wt", [N, O], F16, isOutput=False)
    c_d = nc.declare_dram_parameter("c", [1, O], F32, isOutput=False)
    if nontrivial_ln:
        lnw_d = nc.declare_dram_parameter("lnw", [1, 2, DC], F32, isOutput=False)
        lnbe_d = nc.declare_dram_parameter("lnbe", [O, D], F32, isOutput=False)
    oa_d = nc.declare_dram_parameter("oa", [NA, NPAIR, 2, D], F16, isOutput=True)
    ob_d = nc.declare_dram_parameter("ob", [NB, NPAIR, 2, D], F16, isOutput=True)

    # basis columns for the stats matmuls: ebt[p, b, m] = 1 if m == b else 0
    eb_np = np.broadcast_to(np.eye(GB, dtype=np.float32), (128, GB, GB))

    nsplit = ((0, NA), (NA, NB))

    with tile.TileContext(nc) as tc, ExitStack() as ctx:
        const = ctx.enter_context(tc.tile_pool(name="const", bufs=1))
        xspool = ctx.enter_context(tc.tile_pool(name="xspool", bufs=1))
        xpool = ctx.enter_context(tc.tile_pool(name="xpool", bufs=1))
        wrpool = ctx.enter_context(tc.tile_pool(name="wrpool", bufs=4))
        gpool = ctx.enter_context(tc.tile_pool(name="gpool", bufs=2))
        opool = ctx.enter_context(tc.tile_pool(name="opool", bufs=2))
        small = ctx.enter_context(tc.tile_pool(name="small", bufs=1))
        psum = ctx.enter_context(tc.tile_pool(name="psum", bufs=2, space="PSUM"))

        # ---- loads on the sync queue: the stats operands lead (they gate
        # the PE), then the small constants, then x ----
        ebt_h = nc.inline_tensor(eb_np.astype(ml_dtypes.float8_e4m3), name="ebt")
        eye_h = nc.inline_tensor(np.eye(GB, dtype=np.float32), name="eyeg")
        eye1_h = nc.inline_tensor(np.ones((1, 1), dtype=np.float32), name="eye1")
        ebt = const.tile([128, GB, GB], F8, tag="ebt")
        nc.sync.dma_start(out=ebt[:], in_=ebt_h.ap())
        xst = []
        for g in range(NG):
            t = xspool.tile([128, NDCH, GB, SQ], F8, tag=f"xs{g}", name=f"xs{g}")
            eng = nc.sync if g == 0 else nc.scalar
            for cs in range(0, NDCH, 2):
                eng.dma_start(out=t[:, cs:cs + 2, :, :],
                              in_=xs_d[:, g, cs:cs + 2])
            xst.append(t)
        xa_t = xpool.tile([NA, NPAIR, 2, D], F16, tag="xa")
        xb_t = xpool.tile([NB, NPAIR, 2, D], F16, tag="xb")
        for h in range(2):
            hs = slice(2 * h, 2 * h + 2)
            nc.sync.dma_start(out=xa_t[:, hs, :, :], in_=xa_d[:, hs])
            nc.sync.dma_start(out=xb_t[:, hs, :, :], in_=xb_d[:, hs])
        xtiles = (xa_t, xb_t)
        crow = const.tile([1, O], F32, tag="crow")
        nc.scalar.dma_start(out=crow[:], in_=c_d.ap())
        wt_a = const.tile([NA, O], F16, tag="wt_a")
        wt_b = const.tile([NB, O], F16, tag="wt_b")
        nc.scalar.dma_start(out=wt_a[:], in_=wt_d[0:NA, :])
        nc.scalar.dma_start(out=wt_b[:], in_=wt_d[NA:N, :])
        eyeg = const.tile([GB, GB], F32, tag="eyeg")
        eye1 = const.tile([1, 1], F32, tag="eye1")
        nc.scalar.dma_start(out=eyeg[:], in_=eye_h.ap())
        nc.scalar.dma_start(out=eye1[:], in_=eye1_h.ap())
        if nontrivial_ln:
            lnw_t = const.tile([128, 2, DC], F32, tag="lnw")
            lnw_bcast = bass.AP(tensor=lnw_d.ap().tensor, offset=0,
                                ap=[[0, 128], [DC, 2], [1, DC]])
            nc.scalar.dma_start(out=lnw_t[:], in_=lnw_bcast)
            lnbe_a = const.tile([NA, D], F32, tag="lnbe_a")
            lnbe_b = const.tile([NB, D], F32, tag="lnbe_b")
            nc.scalar.dma_start(out=lnbe_a[:], in_=lnbe_d[0:NA, :])
            nc.scalar.dma_start(out=lnbe_b[:], in_=lnbe_d[NA:O, :])

        # ---- ACT table preload #1 (sqrt set); no DMA dependencies ----
        eps_t = const.tile([GB, 1], F32, tag="eps")
        nc.vector.memset(eps_t[:], LN_EPS)
        zero_t = const.tile([GB, 1], F32, tag="zero")
        nc.vector.memset(zero_t[:], 0.0)
        warm16 = const.tile([128, 128], F16, tag="warm16")
        nc.vector.memset(warm16[:], 0.0)
        scr = small.tile([1, 1], F32, tag="scr")
        nc.scalar.activation(out=scr[:], in_=eps_t[0:1, :],
                             func=AF.Abs_reciprocal_sqrt,
                             bias=eps_t[0:1, :], scale=0.0)

        # ---- PE warm-up (HAM) during the input DMA ----
        warm_ps = psum.tile([GB, SQ], F32, tag="pA", name="warm_ps")
        for w in range(N_WARM):
            nc.tensor.matmul(warm_ps[0:1, 0:128], warm16[:, 0:1],
                             warm16[:, 0:128], start=True, stop=True,
                             skip_group_check=True)

        def emit_stats(g):
            sp = psum.tile([GB, SQ], F32, tag=("pA" if g == 0 else "pB"),
                           name=f"stats{g}")
            for ch in range(NDCH):
                for b in range(GB):
                    nc.tensor.matmul(sp[:], ebt[:, b, :], xst[g][:, ch, b, :],
                                     start=(ch == 0 and b == 0),
                                     stop=(ch == NDCH - 1 and b == GB - 1),
                                     skip_group_check=True)
            return sp

        def emit_varprep(g, sp):
            mu_r = small.tile([GB, N], F32, tag=f"mu{g}", name=f"mu{g}")
            nc.vector.tensor_scalar_mul(out=mu_r[:], in0=sp[:, 0:N],
                                        scalar1=1.0 / D)
            musq_r = small.tile([GB, N], F32, tag=f"musq{g}", name=f"musq{g}")
            nc.vector.tensor_mul(out=musq_r[:], in0=mu_r[:], in1=mu_r[:])
            var_r = small.tile([GB, N], F32, tag=f"var{g}", name=f"var{g}")
            nc.vector.scalar_tensor_tensor(out=var_r[:], in0=sp[:, N:SQ],
                                           scalar=1.0 / D, in1=musq_r[:],
                                           op0=OP.mult, op1=OP.subtract)
            rstd_r = small.tile([GB, N], F32, tag=f"rstd{g}", name=f"rstd{g}")
            ins = nc.scalar.activation(out=rstd_r[:], in_=var_r[:],
                                       func=AF.Abs_reciprocal_sqrt,
                                       bias=eps_t[:], scale=1.0)
            z_r = small.tile([GB, N], F32, tag=f"z{g}", name=f"z{g}")
            nc.vector.tensor_mul(out=z_r[:], in0=mu_r[:], in1=rstd_r[:])
            return rstd_r, z_r, ins

        def emit_prep(g, rstd_r, z_r):
            cols = {}
            for nm, row in (("rstd", rstd_r), ("z", z_r)):
                for ci, (p0, pn) in enumerate(nsplit):
                    ps_t = psum.tile([pn, GB], F32,
                                     tag=("pA" if ci == 0 else "pB"),
                                     name=f"tp_{nm}{ci}_{g}")
                    nc.tensor.transpose(ps_t[:], row[:, p0:p0 + pn], eyeg[:])
                    sb_t = small.tile([pn, GB], F32, tag=f"{nm}T{ci}{g}",
                                      name=f"{nm}T{ci}{g}")
                    nc.vector.tensor_copy(sb_t[:], ps_t[:])
                    cols[(nm, ci)] = sb_t
            def fold_wr(bl):
                bb = g * GB + bl
                wr_pair = []
                for ci, (p0, pn) in enumerate(nsplit):
                    wt_t = wt_a if ci == 0 else wt_b
                    wr = wrpool.tile([pn, O], F16, tag=f"wr{ci}",
                                     name=f"wr{bb}_{ci}")
                    nc.vector.tensor_scalar_mul(
                        out=wr[:], in0=wt_t[:],
                        scalar1=cols[("rstd", ci)][:, bl:bl + 1])
                    wr_pair.append(wr)
                return wr_pair

            wrs = {g * GB: fold_wr(0)}    # first batch's weights lead
            zt16 = []
            for ci, (p0, pn) in enumerate(nsplit):
                z16 = small.tile([pn, GB], F16, tag=f"z16_{ci}{g}",
                                 name=f"z16_{ci}{g}")
                nc.vector.tensor_copy(z16[:], cols[("z", ci)][:])
                zt16.append(z16)
            # gbias[o, b] = c[o] - sum_n wt[n, o] * z[n, b]
            gbias = []
            for oc, (o0, on) in enumerate(nsplit):
                g_ps = psum.tile([on, GB], F32, tag=("pA" if oc == 0 else "pB"),
                                 name=f"g_ps{oc}_{g}")
                for k in range(2):
                    wt_t = wt_a if k == 0 else wt_b
                    nc.tensor.matmul(g_ps[:], wt_t[:, o0:o0 + on], zt16[k][:],
                                     start=(k == 0), stop=(k == 1),
                                     skip_group_check=True)
                gb = small.tile([on, GB], F32, tag=f"gb{oc}{g}",
                                name=f"gb{oc}{g}")
                nc.vector.tensor_scalar(out=gb[:], in0=g_ps[:], scalar1=-1.0,
                                        scalar2=c_cols[oc][:, 0:1],
                                        op0=OP.mult, op1=OP.add)
                gbias.append(gb)
            for bl in range(1, GB):
                wrs[g * GB + bl] = fold_wr(bl)
            return gbias, wrs

        # ---- sequential: all stats, all varprep (rstds back-to-back in one
        # sqrt phase), all prep, then the batch pipeline ----
        sps = [emit_stats(g) for g in range(NG)]
        rz = [emit_varprep(g, sps[g]) for g in range(NG)]

        # c row -> per-partition columns (after stats on the PE queue; the
        # constants land well after the stats operands)
        c_cols = []
        for ci, (p0, pn) in enumerate(nsplit):
            c_ps = psum.tile([pn, 1], F32, tag=("pA" if ci == 0 else "pB"),
                             name=f"ctp{ci}")
            nc.tensor.transpose(c_ps[:], crow[:, p0:p0 + pn], eye1[:])
            c_sb = small.tile([pn, 1], F32, tag=f"c{ci}", name=f"c{ci}")
            nc.vector.tensor_copy(c_sb[:], c_ps[:])
            c_cols.append(c_sb)
        scr2 = small.tile([1, 1], F32, tag="scr2")
        g_pre = nc.scalar.activation(out=scr2[:], in_=eps_t[0:1, :],
                                     func=AF.Gelu, bias=zero_t[0:1, :],
                                     scale=1.0)
        add_dep_helper(g_pre.ins, rz[1][2].ins, sync=True,
                       reason="gelu table after sqrt-set rstds")
        preps = [emit_prep(g, rz[g][0], rz[g][1]) for g in range(NG)]

        for bb in range(BPC):
            g, bl = divmod(bb, GB)
            gbias, wrs = preps[g]
            q, j = divmod(bb, 2)
            if True:
                if j == 0:
                    gts = {}
                for oc, (o0, on) in enumerate(nsplit):
                    pm = psum.tile([on, 2, 512], F32,
                                   tag=("pA" if oc == 0 else "pB"),
                                   name=f"pm{bb}_{oc}")
                    for k, (p0, pn) in enumerate(nsplit):
                        for dc in range(2):
                            nc.tensor.matmul(
                                pm[:, dc, 0:DC],
                                wrs[bb][k][:, o0:o0 + on],
                                xtiles[k][:, q, j, dc * DC:(dc + 1) * DC],
                                start=(k == 0), stop=(k == 1),
                                skip_group_check=True)
                    if nontrivial_ln:
                        lnbe_t = lnbe_a if oc == 0 else lnbe_b
                        nc.vector.tensor_mul(out=pm[:, :, 0:DC],
                                             in0=pm[:, :, 0:DC],
                                             in1=lnw_t[0:on, :, :])
                        nc.vector.tensor_add(
                            out=pm[:, :, 0:DC], in0=pm[:, :, 0:DC],
                            in1=lnbe_t[:, :].rearrange("p (a f) -> p a f", a=2))

                    if j == 0:
                        gt = gpool.tile([on, 2, D], F16, tag=f"g{oc}",
                                        name=f"g{q}_{oc}")
                        gts[oc] = gt
                    gt = gts[oc]
                    nc.scalar.activation(
                        out=gt[:, j, :].rearrange("p (a f) -> p a f", a=2),
                        in_=pm[:, :, 0:DC], func=AF.Gelu,
                        bias=gbias[oc][:, bl:bl + 1], scale=1.0)

                # pair complete: (g+1) in place (4x), multiply by x, store
                if j == 1:
                    for oc, (o0, on) in enumerate(nsplit):
                        nc.vector.tensor_scalar_add(out=gts[oc][:],
                                                    in0=gts[oc][:],
                                                    scalar1=1.0)
                        ot = opool.tile([on, 2, D], F16, tag=f"o{oc}",
                                        name=f"o{q}_{oc}")
                        nc.vector.tensor_mul(out=ot[:], in0=gts[oc][:],
                                             in1=xtiles[oc][:, q, :, :])
                        o_d = oa_d if oc == 0 else ob_d
                        nc.sync.dma_start(out=o_d[:, q], in_=ot[:])

    nc.compile()
    return nc


def kernel(x, ln_w, ln_b, sw, sb, dw, _trace=False):
    import ml_dtypes
    from concourse.bass_utils import run_bass_kernel_spmd

    x = np.asarray(x, dtype=np.float32)
    ln_w = np.asarray(ln_w, dtype=np.float32)
    ln_b = np.asarray(ln_b, dtype=np.float32)
    sw = np.asarray(sw, dtype=np.float32)
    sb = np.asarray(sb, dtype=np.float32)
    dw = np.asarray(dw, dtype=np.float32)

    x16 = x.astype(np.float16)
    # partition-major x: [n-chunk][pair, j, d]
    xr = x16.reshape(N_CORES, NPAIR, 2, N, D)

    # stats operand: [x^T | (x^2)^T] per batch, partition-major, fp8
    xt = x.transpose(0, 2, 1)                       # (B, 768, 196)
    xs = np.concatenate([xt, xt * xt], axis=-1)     # (B, 768, 392)
    # -> [core][128(p), group, d-chunk, batch-in-group, SQ]
    xs = xs.reshape(N_CORES, NG, GB, NDCH, 128, SQ).transpose(0, 4, 1, 3, 2, 5)
    xs8 = np.ascontiguousarray(xs.astype(ml_dtypes.float8_e4m3))

    # Fold dendritic weights into the synapse contraction (host, ~0.1 ms).
    W = np.einsum("om,omn->on", dw, sw)            # (o, n)
    WT = np.ascontiguousarray(W.T.astype(np.float16))
    c = np.einsum("om,om->o", dw, sb.sum(-1)).astype(np.float32)[None, :]

    nontrivial_ln = not (np.all(ln_w == 1.0) and np.all(ln_b == 0.0))
    key = bool(nontrivial_ln)
    if key not in _NC_CACHE:
        _NC_CACHE[key] = _build_nc(nontrivial_ln)
    nc = _NC_CACHE[key]

    in_maps = []
    for i in range(N_CORES):
        xi = xr[i].transpose(2, 0, 1, 3)           # (196, NPAIR, 2, D)
        m = {"xa": np.ascontiguousarray(xi[0:NA]),
             "xb": np.ascontiguousarray(xi[NA:N]),
             "xs": xs8[i], "wt": WT, "c": c}
        if nontrivial_ln:
            m["lnw"] = ln_w.reshape(1, 2, DC)
            m["lnbe"] = (W.sum(-1)[:, None] * ln_b[None, :]).astype(np.float32)
        in_maps.append(m)

    res = run_bass_kernel_spmd(nc, in_maps, core_ids=list(range(N_CORES)),
                               trace=_trace)
    out = np.empty((B, N, D), dtype=np.float16)
    outr = out.reshape(N_CORES, NPAIR, 2, N, D)
    for i in range(N_CORES):
        oa = res.results[i]["oa"]                  # (NA, NPAIR, 2, D)
        ob = res.results[i]["ob"]
        outr[i, :, :, 0:NA] = oa.transpose(1, 2, 0, 3)
        outr[i, :, :, NA:N] = ob.transpose(1, 2, 0, 3)
    out = out.astype(np.float32)
    if _trace:
        return out, res
    return out


# revision 38
# speedup vs baseline: 1.0505x; 1.0505x over previous
"""Trainium2 Bass kernel for nn_DNM_Conv (LayerNorm -> synapse contraction ->
dendritic weighting -> GELU -> residual multiply).

Algebraic reduction of the reference:
    y = LayerNorm(x)                                  (b, n, d)
    t[b,o,d] = sum_n W[o,n] * y[b,n,d] + c[o]
        where W[o,n] = sum_m dw[o,m]*sw[o,m,n],  c[o] = sum_{m,n} dw[o,m]*sb[o,m,n]
    out = x * (gelu_erf(t) + 1)                       (o == n == 196)

Kernel structure (v5):
  * All inputs partition-major; loads ordered tiny-first on the sync queue
    so constants don't queue behind megabyte transfers; stores also on the
    sync HW queue (the gpsimd software queue moves small packets slowly).
  * LN stats via PE: host ships [x^T | (x^2)^T] fp8e4; basis-column matmuls
    reduce over d into [sum x | sum x^2] PSUM rows, two batch-groups of 4.
    Dummy matmuls warm the PE (HAM) during the input DMA.
  * mean/var on [4,196] rows (DVE), both rstd ops back-to-back in one
    sqrt table phase (ACT), rows PE-transposed to per-partition columns.
  * rstd folded into per-batch weights wr = wt * rstd (DVE tensor_scalar);
    the mu term becomes a per-batch gelu bias c - (W @ (rstd*mu)) computed
    by a tiny PE matmul against the transposed z columns.
  * GELU reads both d-chunk PSUM banks of a batch in one instruction.
  * Output: (g+1) in place (DVE 4x) then one tensor_tensor multiply by x
    per (pair, o-chunk); single DRAM write, no residual seed/accumulate.
  * c shipped as a [1, O] row (one DMA descriptor) and PE-transposed.

Distribution: data-parallel over batch, 8 batches per core on 8 cores.
Datapath fp16 (host casts), fp8 stats operands, fp32 PSUM + LN statistics.
"""

import numpy as np

B, N, D, O, M = 64, 196, 768, 196, 2
N_CORES = 8
BPC = B // N_CORES          # batches per core
NPAIR = BPC // 2            # batch pairs
NG = 2                      # stats batch groups
GB = BPC // NG              # batches per group (4)
NA, NB = 128, 68            # n (and o) partition split
DC = 384                    # matmul free-dim chunk (one PSUM bank)
NDCH = D // 128             # d-chunks for the stats matmuls (6)
SQ = 2 * N                  # stats row width: [sum x | sum x^2]
LN_EPS = 1e-5
N_WARM = 18                 # PE warm-up dummy matmuls

_NC_CACHE = {}


def _build_nc(nontrivial_ln):
    import concourse.bacc as bacc
    import concourse.tile as tile
    import concourse.bass as bass
    from concourse.tile import add_dep_helper
    from concourse import mybir
    from contextlib import ExitStack
    import ml_dtypes

    F32 = mybir.dt.float32
    F16 = mybir.dt.float16
    F8 = mybir.dt.float8e4
    AF = mybir.ActivationFunctionType
    OP = mybir.AluOpType

    nc = bacc.Bacc()
    xa_d = nc.declare_dram_parameter("xa", [NA, NPAIR, 2, D], F16, isOutput=False)
    xb_d = nc.declare_dram_parameter("xb", [NB, NPAIR, 2, D], F16, isOutput=False)
    xs_d = nc.declare_dram_parameter("xs", [128, NG, NDCH, GB, SQ], F8,
                                     isOutput=False)
    wt_d = nc.declare_dram_parameter("wt", [N, O], F16, isOutput=False)
    c_d = nc.declare_dram_parameter("c", [1, O], F32, isOutput=False)
    if nontrivial_ln:
        lnw_d = nc.declare_dram_parameter("lnw", [1, 2, DC], F32, isOutput=False)
        lnbe_d = nc.declare_dram_parameter("lnbe", [O, D], F32, isOutput=False)
    oa_d = nc.declare_dram_parameter("oa", [NA, NPAIR, 2, D], F16, isOutput=True)
    ob_d = nc.declare_dram_parameter("ob", [NB, NPAIR, 2, D], F16, isOutput=True)

    # basis columns for the stats matmuls: ebt[p, b, m] = 1 if m == b else 0
    eb_np = np.broadcast_to(np.eye(GB, dtype=np.float32), (128, GB, GB))

    nsplit = ((0, NA), (NA, NB))

    with tile.TileContext(nc) as tc, ExitStack() as ctx:
        const = ctx.enter_context(tc.tile_pool(name="const", bufs=1))
        xspool = ctx.enter_context(tc.tile_pool(name="xspool", bufs=1))
        xpool = ctx.enter_context(tc.tile_pool(name="xpool", bufs=1))
        wrpool = ctx.enter_context(tc.tile_pool(name="wrpool", bufs=4))
        gpool = ctx.enter_context(tc.tile_pool(name="gpool", bufs=2))
        opool = ctx.enter_context(tc.tile_pool(name="opool", bufs=2))
        small = ctx.enter_context(tc.tile_pool(name="small", bufs=1))
        psum = ctx.enter_context(tc.tile_pool(name="psum", bufs=2, space="PSUM"))

        # ---- loads on the sync queue: the stats operands lead (they gate
        # the PE), then the small constants, then x ----
        ebt_h = nc.inline_tensor(eb_np.astype(ml_dtypes.float8_e4m3), name="ebt")
        eye_h = nc.inline_tensor(np.eye(GB, dtype=np.float32), name="eyeg")
        eye1_h = nc.inline_tensor(np.ones((1, 1), dtype=np.float32), name="eye1")
        ebt = const.tile([128, GB, GB], F8, tag="ebt")
        nc.sync.dma_start(out=ebt[:], in_=ebt_h.ap())
        xst = []
        for g in range(NG):
            t = xspool.tile([128, NDCH, GB, SQ], F8, tag=f"xs{g}", name=f"xs{g}")
            for cs in range(0, NDCH, 2):
                nc.sync.dma_start(out=t[:, cs:cs + 2, :, :],
                                  in_=xs_d[:, g, cs:cs + 2])
            xst.append(t)
        crow = const.tile([1, O], F32, tag="crow")
        nc.sync.dma_start(out=crow[:], in_=c_d.ap())
        wt_a = const.tile([NA, O], F16, tag="wt_a")
        wt_b = const.tile([NB, O], F16, tag="wt_b")
        nc.sync.dma_start(out=wt_a[:], in_=wt_d[0:NA, :])
        nc.sync.dma_start(out=wt_b[:], in_=wt_d[NA:N, :])
        eyeg = const.tile([GB, GB], F32, tag="eyeg")
        eye1 = const.tile([1, 1], F32, tag="eye1")
        nc.sync.dma_start(out=eyeg[:], in_=eye_h.ap())
        nc.sync.dma_start(out=eye1[:], in_=eye1_h.ap())
        if nontrivial_ln:
            lnw_t = const.tile([128, 2, DC], F32, tag="lnw")
            lnw_bcast = bass.AP(tensor=lnw_d.ap().tensor, offset=0,
                                ap=[[0, 128], [DC, 2], [1, DC]])
            nc.sync.dma_start(out=lnw_t[:], in_=lnw_bcast)
            lnbe_a = const.tile([NA, D], F32, tag="lnbe_a")
            lnbe_b = const.tile([NB, D], F32, tag="lnbe_b")
            nc.sync.dma_start(out=lnbe_a[:], in_=lnbe_d[0:NA, :])
            nc.sync.dma_start(out=lnbe_b[:], in_=lnbe_d[NA:O, :])
        xa_t = xpool.tile([NA, NPAIR, 2, D], F16, tag="xa")
        xb_t = xpool.tile([NB, NPAIR, 2, D], F16, tag="xb")
        for h in range(2):
            hs = slice(2 * h, 2 * h + 2)
            nc.sync.dma_start(out=xa_t[:, hs, :, :], in_=xa_d[:, hs])
            nc.sync.dma_start(out=xb_t[:, hs, :, :], in_=xb_d[:, hs])
        xtiles = (xa_t, xb_t)

        # ---- ACT table preload #1 (sqrt set); no DMA dependencies ----
        eps_t = const.tile([GB, 1], F32, tag="eps")
        nc.vector.memset(eps_t[:], LN_EPS)
        zero_t = const.tile([GB, 1], F32, tag="zero")
        nc.vector.memset(zero_t[:], 0.0)
        warm16 = const.tile([128, 128], F16, tag="warm16")
        nc.vector.memset(warm16[:], 0.0)
        scr = small.tile([1, 1], F32, tag="scr")
        nc.scalar.activation(out=scr[:], in_=eps_t[0:1, :],
                             func=AF.Abs_reciprocal_sqrt,
                             bias=eps_t[0:1, :], scale=0.0)

        # ---- PE warm-up (HAM) during the input DMA ----
        warm_ps = psum.tile([GB, SQ], F32, tag="pA", name="warm_ps")
        for w in range(N_WARM):
            nc.tensor.matmul(warm_ps[0:1, 0:128], warm16[:, 0:1],
                             warm16[:, 0:128], start=True, stop=True,
                             skip_group_check=True)

        def emit_stats(g):
            sp = psum.tile([GB, SQ], F32, tag=("pA" if g == 0 else "pB"),
                           name=f"stats{g}")
            for ch in range(NDCH):
                for b in range(GB):
                    nc.tensor.matmul(sp[:], ebt[:, b, :], xst[g][:, ch, b, :],
                                     start=(ch == 0 and b == 0),
                                     stop=(ch == NDCH - 1 and b == GB - 1),
                                     skip_group_check=True)
            return sp

        def emit_varprep(g, sp):
            mu_r = small.tile([GB, N], F32, tag=f"mu{g}", name=f"mu{g}")
            nc.vector.tensor_scalar_mul(out=mu_r[:], in0=sp[:, 0:N],
                                        scalar1=1.0 / D)
            musq_r = small.tile([GB, N], F32, tag=f"musq{g}", name=f"musq{g}")
            nc.vector.tensor_mul(out=musq_r[:], in0=mu_r[:], in1=mu_r[:])
            var_r = small.tile([GB, N], F32, tag=f"var{g}", name=f"var{g}")
            nc.vector.scalar_tensor_tensor(out=var_r[:], in0=sp[:, N:SQ],
                                           scalar=1.0 / D, in1=musq_r[:],
                                           op0=OP.mult, op1=OP.subtract)
            rstd_r = small.tile([GB, N], F32, tag=f"rstd{g}", name=f"rstd{g}")
            ins = nc.scalar.activation(out=rstd_r[:], in_=var_r[:],
                                       func=AF.Abs_reciprocal_sqrt,
                                       bias=eps_t[:], scale=1.0)
            z_r = small.tile([GB, N], F32, tag=f"z{g}", name=f"z{g}")
            nc.vector.tensor_mul(out=z_r[:], in0=mu_r[:], in1=rstd_r[:])
            return rstd_r, z_r, ins

        def emit_prep(g, rstd_r, z_r):
            cols = {}
            for nm, row in (("rstd", rstd_r), ("z", z_r)):
                for ci, (p0, pn) in enumerate(nsplit):
                    ps_t = psum.tile([pn, GB], F32,
                                     tag=("pA" if ci == 0 else "pB"),
                                     name=f"tp_{nm}{ci}_{g}")
                    nc.tensor.transpose(ps_t[:], row[:, p0:p0 + pn], eyeg[:])
                    sb_t = small.tile([pn, GB], F32, tag=f"{nm}T{ci}{g}",
                                      name=f"{nm}T{ci}{g}")
                    nc.vector.tensor_copy(sb_t[:], ps_t[:])
                    cols[(nm, ci)] = sb_t
            def fold_wr(bl):
                bb = g * GB + bl
                wr_pair = []
                for ci, (p0, pn) in enumerate(nsplit):
                    wt_t = wt_a if ci == 0 else wt_b
                    wr = wrpool.tile([pn, O], F16, tag=f"wr{ci}",
                                     name=f"wr{bb}_{ci}")
                    nc.vector.tensor_scalar_mul(
                        out=wr[:], in0=wt_t[:],
                        scalar1=cols[("rstd", ci)][:, bl:bl + 1])
                    wr_pair.append(wr)
                return wr_pair

            wrs = {g * GB: fold_wr(0)}    # first batch's weights lead
            zt16 = []
            for ci, (p0, pn) in enumerate(nsplit):
                z16 = small.tile([pn, GB], F16, tag=f"z16_{ci}{g}",
                                 name=f"z16_{ci}{g}")
                nc.vector.tensor_copy(z16[:], cols[("z", ci)][:])
                zt16.append(z16)
            # gbias[o, b] = c[o] - sum_n wt[n, o] * z[n, b]
            gbias = []
            for oc, (o0, on) in enumerate(nsplit):
                g_ps = psum.tile([on, GB], F32, tag=("pA" if oc == 0 else "pB"),
                                 name=f"g_ps{oc}_{g}")
                for k in range(2):
                    wt_t = wt_a if k == 0 else wt_b
                    nc.tensor.matmul(g_ps[:], wt_t[:, o0:o0 + on], zt16[k][:],
                                     start=(k == 0), stop=(k == 1),
                                     skip_group_check=True)
                gb = small.tile([on, GB], F32, tag=f"gb{oc}{g}",
                                name=f"gb{oc}{g}")
                nc.vector.tensor_scalar(out=gb[:], in0=g_ps[:], scalar1=-1.0,
                                        scalar2=c_cols[oc][:, 0:1],
                                        op0=OP.mult, op1=OP.add)
                gbias.append(gb)
            for bl in range(1, GB):
                wrs[g * GB + bl] = fold_wr(bl)
            return gbias, wrs

        # ---- sequential: all stats, all varprep (rstds back-to-back in one
        # sqrt phase), all prep, then the batch pipeline ----
        sps = [emit_stats(g) for g in range(NG)]
        rz = [emit_varprep(g, sps[g]) for g in range(NG)]

        # filler matmuls bridge the stats->prep gap so HAM stays at 8/8
        for w in range(14):
            nc.tensor.matmul(warm_ps[0:1, 0:128], warm16[:, 0:1],
                             warm16[:, 0:128], start=True, stop=True,
                             skip_group_check=True)

        # c row -> per-partition columns (after stats on the PE queue; the
        # constants land well after the stats operands)
        c_cols = []
        for ci, (p0, pn) in enumerate(nsplit):
            c_ps = psum.tile([pn, 1], F32, tag=("pA" if ci == 0 else "pB"),
                             name=f"ctp{ci}")
            nc.tensor.transpose(c_ps[:], crow[:, p0:p0 + pn], eye1[:])
            c_sb = small.tile([pn, 1], F32, tag=f"c{ci}", name=f"c{ci}")
            nc.vector.tensor_copy(c_sb[:], c_ps[:])
            c_cols.append(c_sb)
        scr2 = small.tile([1, 1], F32, tag="scr2")
        g_pre = nc.scalar.activation(out=scr2[:], in_=eps_t[0:1, :],
                                     func=AF.Gelu, bias=zero_t[0:1, :],
                                     scale=1.0)
        add_dep_helper(g_pre.ins, rz[1][2].ins, sync=True,
                       reason="gelu table after sqrt-set rstds")
        preps = [emit_prep(g, rz[g][0], rz[g][1]) for g in range(NG)]

        for bb in range(BPC):
            g, bl = divmod(bb, GB)
            gbias, wrs = preps[g]
            q, j = divmod(bb, 2)
            if True:
                if j == 0:
                    gts = {}
                for oc, (o0, on) in enumerate(nsplit):
                    pm = psum.tile([on, 2, 512], F32,
                                   tag=("pA" if oc == 0 else "pB"),
                                   name=f"pm{bb}_{oc}")
                    for k, (p0, pn) in enumerate(nsplit):
                        for dc in range(2):
                            nc.tensor.matmul(
                                pm[:, dc, 0:DC],
                                wrs[bb][k][:, o0:o0 + on],
                                xtiles[k][:, q, j, dc * DC:(dc + 1) * DC],
                                start=(k == 0), stop=(k == 1),
                                skip_group_check=True)
                    if nontrivial_ln:
                        lnbe_t = lnbe_a if oc == 0 else lnbe_b
                        nc.vector.tensor_mul(out=pm[:, :, 0:DC],
                                             in0=pm[:, :, 0:DC],
                                             in1=lnw_t[0:on, :, :])
                        nc.vector.tensor_add(
                            out=pm[:, :, 0:DC], in0=pm[:, :, 0:DC],
                            in1=lnbe_t[:, :].rearrange("p (a f) -> p a f", a=2))

                    if j == 0:
                        gt = gpool.tile([on, 2, D], F16, tag=f"g{oc}",
                                        name=f"g{q}_{oc}")
                        gts[oc] = gt
                    gt = gts[oc]
                    nc.scalar.activation(
                        out=gt[:, j, :].rearrange("p (a f) -> p a f", a=2),
                        in_=pm[:, :, 0:DC], func=AF.Gelu,
                        bias=gbias[oc][:, bl:bl + 1], scale=1.0)

                # pair complete: (g+1) in place (4x), multiply by x, store
                if j == 1:
                    for oc, (o0, on) in enumerate(nsplit):
                        nc.vector.tensor_scalar_add(out=gts[oc][:],
                                                    in0=gts[oc][:],
                                                    scalar1=1.0)
                        ot = opool.tile([on, 2, D], F16, tag=f"o{oc}",
                                        name=f"o{q}_{oc}")
                        nc.vector.tensor_mul(out=ot[:], in0=gts[oc][:],
                                             in1=xtiles[oc][:, q, :, :])
                        o_d = oa_d if oc == 0 else ob_d
                        nc.sync.dma_start(out=o_d[:, q], in_=ot[:])

    nc.compile()
    return nc


def kernel(x, ln_w, ln_b, sw, sb, dw, _trace=False):
    import ml_dtypes
    from concourse.bass_utils import run_bass_kernel_spmd

    x = np.asarray(x, dtype=np.float32)
    ln_w = np.asarray(ln_w, dtype=np.float32)
    ln_b = np.asarray(ln_b, dtype=np.float32)
    sw = np.asarray(sw, dtype=np.float32)
    sb = np.asarray(sb, dtype=np.float32)
    dw = np.asarray(dw, dtype=np.float32)

    x16 = x.astype(np.float16)
    # partition-major x: [n-chunk][pair, j, d]
    xr = x16.reshape(N_CORES, NPAIR, 2, N, D)

    # stats operand: [x^T | (x^2)^T] per batch, partition-major, fp8
    xt = x.transpose(0, 2, 1)                       # (B, 768, 196)
    xs = np.concatenate([xt, xt * xt], axis=-1)     # (B, 768, 392)
    # -> [core][128(p), group, d-chunk, batch-in-group, SQ]
    xs = xs.reshape(N_CORES, NG, GB, NDCH, 128, SQ).transpose(0, 4, 1, 3, 2, 5)
    xs8 = np.ascontiguousarray(xs.astype(ml_dtypes.float8_e4m3))

    # Fold dendritic weights into the synapse contraction (host, ~0.1 ms).
    W = np.einsum("om,omn->on", dw, sw)            # (o, n)
    WT = np.ascontiguousarray(W.T.astype(np.float16))
    c = np.einsum("om,om->o", dw, sb.sum(-1)).astype(np.float32)[None, :]

    nontrivial_ln = not (np.all(ln_w == 1.0) and np.all(ln_b == 0.0))
    key = bool(nontrivial_ln)
    if key not in _NC_CACHE:
        _NC_CACHE[key] = _build_nc(nontrivial_ln)
    nc = _NC_CACHE[key]

    in_maps = []
    for i in range(N_CORES):
        xi = xr[i].transpose(2, 0, 1, 3)           # (196, NPAIR, 2, D)
        m = {"xa": np.ascontiguousarray(xi[0:NA]),
             "xb": np.ascontiguousarray(xi[NA:N]),
             "xs": xs8[i], "wt": WT, "c": c}
        if nontrivial_ln:
            m["lnw"] = ln_w.reshape(1, 2, DC)
            m["lnbe"] = (W.sum(-1)[:, None] * ln_b[None, :]).astype(np.float32)
        in_maps.append(m)

    res = run_bass_kernel_spmd(nc, in_maps, core_ids=list(range(N_CORES)),
                               trace=_trace)
    out = np.empty((B, N, D), dtype=np.float16)
    outr = out.reshape(N_CORES, NPAIR, 2, N, D)
    for i in range(N_CORES):
        oa = res.results[i]["oa"]                  # (NA, NPAIR, 2, D)
        ob = res.results[i]["ob"]
        outr[i, :, :, 0:NA] = oa.transpose(1, 2, 0, 3)
        outr[i, :, :, NA:N] = ob.transpose(1, 2, 0, 3)
    out = out.astype(np.float32)
    if _trace:
        return out, res
    return out
